# revision 1
# baseline (speedup 1.0000x reference)
"""Multi-head graph attention (GAT) Trainium2 kernel.

Row-sharded across 8 NeuronCores: core i owns queries [i*1024, (i+1)*1024).

Math (per head h, with Wh = h @ W_h, a = Wh@a1, b = Wh@a2):
    e[i,j]  = leakyrelu(a_i + b_j, 0.2)
    attn    = softmax_j(where(adj>0, e, -9e15))
    out_h   = elu(attn @ Wh)
    out     = concat_h(out_h) @ Wp.T + bp

Key factorization used on-chip (exact):
    exp(lrelu(s)) = exp(0.2 s) * max(exp(0.8 s), 1)
                  = (ea02_i * vb02_j) * max(ea08_i * vb08_j, 1)
so the masked-exp score matrix is built with 3 DVE ops per tile (no ACT
on the big matrix), and softmax needs no row-max subtraction (|s| <~ 25
so exp stays in fp32/bf16 range).

The PV contraction runs on the TensorEngine with keys on partitions, so
adj must be transposed on load: adj int32 holds only 0/1, so we view it
as int16 pairs and DMA-transpose (2-byte only) the low halves (stride-2
source AP) -- same HBM lines, no extra traffic.
"""

import os
from contextlib import ExitStack

import numpy as np

import concourse.bacc as bacc
import concourse.bass as bass
import concourse.mybir as mybir
import concourse.tile as tile

F32 = mybir.dt.float32
BF16 = mybir.dt.bfloat16
I32 = mybir.dt.int32
I16 = mybir.dt.int16

ALU = mybir.AluOpType
AF = mybir.ActivationFunctionType

N = 8192          # nodes
IN_F = 256        # input features
H = 4             # heads
DH = 64           # head dim
NCORES = 8
QN = N // NCORES  # queries per core (1024)
KB = N // 128     # key blocks of 128 (64)
QH = QN // 512    # 512-wide query halves per core (2)


def build_nc():
    nc = bacc.Bacc("TRN2", target_bir_lowering=False, debug=False)

    ht = nc.declare_dram_parameter("ht", [IN_F, N], F32, False)       # h.T (replicated)
    hqt = nc.declare_dram_parameter("hqt", [IN_F, QN], F32, False)    # h.T query slice
    adjq = nc.declare_dram_parameter("adjq", [QN, N], I16, False)    # adj row shard (0/1 as i16)
    wam = nc.declare_dram_parameter("wam", [IN_F, IN_F + 8], F32, False)  # [W_all | a1~ | a2~]
    wpt = nc.declare_dram_parameter("wpt", [IN_F, IN_F], F32, False)  # Wp.T
    bp = nc.declare_dram_parameter("bp", [IN_F], F32, False)
    out = nc.declare_dram_parameter("out", [QN, IN_F], F32, True)

    with ExitStack() as ctx:
        tc = ctx.enter_context(tile.TileContext(nc))

        persist = ctx.enter_context(tc.tile_pool(name="persist", bufs=1))
        # Values+denominator stationaries: [k-part, kblock, head, dh+1];
        # written unscaled [Wh | 1] in setup, scaled in place by vb02 lazily
        # in the main loop.
        whv = persist.tile([128, KB, H, DH + 1], BF16)
        # per-key exp factors (per-partition scalars for the main loop)
        vb02 = persist.tile([128, H, KB], F32)
        vb08 = persist.tile([128, H, KB], F32)
        # per-query exp factor broadcast across partitions (0.8-branch only;
        # the 0.2-branch per-query factor cancels in softmax normalization)
        ea08b = persist.tile([128, H, QN], BF16)
        wpt_sb = persist.tile([128, 2, IN_F], F32)
        bpb = persist.tile([128, IN_F], F32)
        ones1 = persist.tile([1, 128], BF16)
        ones_f = persist.tile([1, 64], F32)

        # Main-loop pool created and slot-pinned BEFORE setup so its tiles
        # never share SBUF with setup tiles (whose last readers are late
        # setup matmuls -- sharing would gate the mask pipeline on them).
        MBUFS = int(os.environ.get("GAT_BUFS", "4"))
        mloop = ctx.enter_context(tc.tile_pool(name="mloop", bufs=MBUFS))
        for _b in range(MBUFS):
            _t = mloop.tile([128, QN], I16, tag="adjT")
            nc.vector.memset(_t[0:1, 0:2], 0)
            _t = mloop.tile([128, QN], BF16, tag="mT")
            nc.vector.memset(_t[0:1, 0:2], 0.0)
            _t = mloop.tile([128, 2, QN], BF16, tag="g")
            nc.vector.memset(_t[0:1, 0, 0:2], 0.0)
            _t = mloop.tile([128, 2, QN], BF16, tag="pm")
            nc.vector.memset(_t[0:1, 0, 0:2], 0.0)

        # ---------------- setup phase ----------------
        with tc.tile_pool(name="setup", bufs=1) as setup, \
             tc.tile_pool(name="htp", bufs=4) as htp, \
             tc.tile_pool(name="spsum", bufs=4, space="PSUM") as spsum, \
             tc.tile_pool(name="spsum2", bufs=2, space="PSUM") as spsum2:
            nc.vector.memset(ones1, 1.0)
            nc.vector.memset(ones_f, 1.0)


            wam_sb = setup.tile([128, 2, IN_F + 8], F32)
            nc.scalar.dma_start(wam_sb, wam[:, :].rearrange("(c p) w -> p c w", p=128))
            nc.scalar.dma_start(wpt_sb, wpt[:, :].rearrange("(c p) w -> p c w", p=128))
            bp_ap = bp[:]
            nc.gpsimd.dma_start(bpb, bass.AP(tensor=bp_ap.tensor, offset=bp_ap.offset,
                                             ap=[[0, 128]] + list(bp_ap.ap)))

            hqt_sb = setup.tile([128, 2, QN], F32)
            nc.scalar.dma_start(hqt_sb, hqt[:, :].rearrange("(c p) n -> p c n", p=128))

            # a-scores first (needs only hqt): exp rows -> broadcast tiles so
            # the main loop's mask chain starts as early as possible.
            ea08r = setup.tile([1, H, QN], BF16)
            for h in range(H):
                for qh in range(QH):
                    qsl = slice(qh * 512, (qh + 1) * 512)
                    pa = spsum2.tile([1, 512], F32, tag="a_ps")
                    nc.tensor.matmul(pa, wam_sb[:, 0, IN_F + h:IN_F + h + 1],
                                     hqt_sb[:, 0, qsl], start=True, stop=False)
                    nc.tensor.matmul(pa, wam_sb[:, 1, IN_F + h:IN_F + h + 1],
                                     hqt_sb[:, 1, qsl], start=False, stop=True)
                    nc.scalar.activation(ea08r[:, h, qsl], pa, AF.Exp, scale=0.8)
                    pb2 = spsum2.tile([128, 512], F32, tag="b_ps")
                    nc.tensor.matmul(pb2, ones1, ea08r[:, h, qsl])
                    nc.scalar.copy(ea08b[:, h, qsl], pb2)

            nc.vector.memset(whv[:, :, :, DH:DH + 1], 1.0)
            # Wh (natural [k, h*dh]) and b-scores per key chunk. ht is
            # streamed in quarters through a small tile so the whole setup +
            # main-loop working set fits SBUF concurrently (no allocator
            # waits gating the mask pipeline). PSUM slots drain via two fast
            # DVE copies; exp factors computed from the SBUF copy per
            # 16-chunk group.
            ht_r = ht[:, :].rearrange("(c p) n -> p c n", p=128)
            for i in range(4):
                htq = htp.tile([128, 2, N // 4], F32, tag="htq")
                nsl = slice(i * (N // 4), (i + 1) * (N // 4))
                nc.scalar.dma_start(htq, ht_r[:, :, nsl])
                for kq in range(16):
                    kc = i * 16 + kq
                    ps = spsum.tile([128, IN_F + 8], F32, tag="wh_ps")
                    ksl = slice(kq * 128, (kq + 1) * 128)
                    nc.tensor.matmul(ps, htq[:, 0, ksl], wam_sb[:, 0, :],
                                     start=True, stop=False)
                    nc.tensor.matmul(ps, htq[:, 1, ksl], wam_sb[:, 1, :],
                                     start=False, stop=True)
                    bsc = ps[:, IN_F + 4:IN_F + 8].rearrange("p (h o) -> p h o", o=1)
                    nc.scalar.activation(vb02[:, :, kc:kc + 1], bsc,
                                         AF.Exp, scale=0.2)
                    nc.scalar.activation(vb08[:, :, kc:kc + 1], bsc,
                                         AF.Exp, scale=0.8)
                    if kc % 2 == 0:
                        nc.scalar.copy(
                            whv[:, kc, :, 0:DH],
                            ps[:, 0:IN_F].rearrange("p (h d) -> p h d", h=H))
                    else:
                        nc.vector.tensor_copy(
                            whv[:, kc, :, 0:DH],
                            ps[:, 0:IN_F].rearrange("p (h d) -> p h d", h=H))


        # ---------------- main loop ----------------

        tailp = ctx.enter_context(tc.tile_pool(name="tailp", bufs=1))
        denr = tailp.tile([1, H, QN], F32)
        gfin = tailp.tile([128, 2, QN], F32)

        mpsum_cm = tc.tile_pool(name="mpsum", bufs=1, space="PSUM")
        mpsum = mpsum_cm.__enter__()
        acc = mpsum.tile([DH + 1, H, QH, 512], F32)

        # engine-split fractions (tunable): portion of pm TT pair-ops on
        # GPSIMD and of mask casts on ACT.
        POOL_TT = int(os.environ.get("GAT_POOL_TT", "42"))    # of 128 pm pair-ops
        ACT_CAST = int(os.environ.get("GAT_ACT_CAST", "64"))  # of 64 casts

        ti = 0
        ci = 0

        def frac_hit(i, frac, tot):
            return (i * frac) // tot != ((i - 1) * frac) // tot

        for kb in range(KB):
            # lazy in-place vb02 scaling of this block's stationary
            for h in range(H):
                nc.vector.tensor_scalar(
                    whv[:, kb, h, :], whv[:, kb, h, :],
                    vb02[:, h, kb:kb + 1], None, op0=ALU.mult)
            at = mloop.tile([128, QN], I16, tag="adjT")
            nc.sync.dma_start_transpose(at, adjq[:, kb * 128:(kb + 1) * 128])
            mt = mloop.tile([128, QN], BF16, tag="mT")
            ci += 1
            if frac_hit(ci, ACT_CAST, 64):
                nc.scalar.copy(mt, at)
            else:
                nc.vector.tensor_copy(mt, at)
            # mask AP read twice along a step-0 middle dim for head pairs
            mt2 = bass.AP(tensor=mt.tensor, offset=mt.offset,
                          ap=[list(mt.ap[0]), [0, 2], list(mt.ap[1])])
            for hp in range(H // 2):  # head pairs
                g2 = mloop.tile([128, 2, QN], BF16, tag="g")
                for j in range(2):
                    h = hp * 2 + j
                    nc.vector.tensor_scalar(
                        g2[:, j, :], ea08b[:, h, :], vb08[:, h, kb:kb + 1], 1.0,
                        op0=ALU.mult, op1=ALU.max)
                pm2 = mloop.tile([128, 2, QN], BF16, tag="pm")
                ti += 1
                if frac_hit(ti, POOL_TT, 128):
                    nc.gpsimd.tensor_mul(pm2, g2, mt2)
                else:
                    nc.vector.tensor_mul(pm2, g2, mt2)
                for j in range(2):
                    h = hp * 2 + j
                    for qh in range(QH):
                        nc.tensor.matmul(acc[:, h, qh, :], whv[:, kb, h, :],
                                         pm2[:, j, qh * 512:(qh + 1) * 512],
                                         start=(kb == 0), stop=(kb == KB - 1))

        # ---------------- tail: normalize, elu, out-proj ----------------
        graw = tailp.tile([128, 2, QN], F32)
        for h in range(H):
            for qh in range(QH):
                qsl = slice(qh * 512, (qh + 1) * 512)
                nc.vector.reciprocal(denr[:, h, qsl], acc[DH:DH + 1, h, qh, :])
            # raw (unnormalized) h'.T for head h -> partitions [(h%2)*64, ...)
            nc.scalar.copy(
                graw[(h % 2) * 64:(h % 2) * 64 + 64, h // 2, :],
                acc[0:DH, h, :, :].rearrange("p a b -> p (a b)"))
        mpsum_cm.__exit__(None, None, None)

        with tc.tile_pool(name="tpsum", bufs=2, space="PSUM") as tpsum:
            # normalize: broadcast 1/den across partitions via ones-matmul
            for j in range(2):
                for qh in range(QH):
                    qsl = slice(qh * 512, (qh + 1) * 512)
                    rps = tpsum.tile([128, 512], F32, tag="r_ps")
                    nc.tensor.matmul(rps[0:64, :], ones_f, denr[:, 2 * j, qsl])
                    nc.tensor.matmul(rps[64:128, :], ones_f, denr[:, 2 * j + 1, qsl])
                    nc.vector.tensor_mul(gfin[:, j, qsl], graw[:, j, qsl], rps)

            # elu(x) = relu(x) + exp(min(x, 0)) - 1, per quarter so the
            # out-projection can start on finished columns early
            for j in range(2):
                for qh in range(QH):
                    qsl = slice(qh * 512, (qh + 1) * 512)
                    t = tailp.tile([128, 512], F32, tag="elu_t")
                    nc.vector.tensor_scalar(t, gfin[:, j, qsl], 0.0, None,
                                            op0=ALU.min)
                    e = tailp.tile([128, 512], F32, tag="elu_e")
                    nc.scalar.activation(e, t, AF.Exp)
                    em1 = tailp.tile([128, 512], F32, tag="elu_em1")
                    nc.vector.tensor_scalar(em1, e, -1.0, None, op0=ALU.add)
                    nc.vector.scalar_tensor_tensor(gfin[:, j, qsl], gfin[:, j, qsl],
                                                   0.0, em1, op0=ALU.max, op1=ALU.add)

            for qc in range(QN // 128):
                qsl = slice(qc * 128, (qc + 1) * 128)
                po = tpsum.tile([128, IN_F], F32, tag="out_ps")
                nc.tensor.matmul(po, gfin[:, 0, qsl], wpt_sb[:, 0, :],
                                 start=True, stop=False)
                nc.tensor.matmul(po, gfin[:, 1, qsl], wpt_sb[:, 1, :],
                                 start=False, stop=True)
                fin = tailp.tile([128, IN_F], F32, tag="fin")
                nc.vector.scalar_tensor_tensor(fin, po, 0.0, bpb,
                                               op0=ALU.add, op1=ALU.add)
                nc.sync.dma_start(out[qsl, :], fin)

    nc.compile()
    return nc


_NC_CACHE = {}
LAST_RESULTS = None


def _get_nc():
    if "nc" not in _NC_CACHE:
        _NC_CACHE["nc"] = build_nc()
    return _NC_CACHE["nc"]


def kernel(h, adj, W, a1, a2, Wp, bp):
    from concourse.bass_utils import run_bass_kernel_spmd

    h = np.asarray(h, dtype=np.float32)
    adj = np.asarray(adj)
    W = np.asarray(W, dtype=np.float32)
    a1 = np.asarray(a1, dtype=np.float32)
    a2 = np.asarray(a2, dtype=np.float32)
    Wp = np.asarray(Wp, dtype=np.float32)
    bp = np.asarray(bp, dtype=np.float32)

    # host-side parameter marshaling
    W_all = np.ascontiguousarray(W.transpose(1, 0, 2).reshape(IN_F, H * DH))
    amat_a = np.einsum("hid,hd->ih", W, a1)  # [256, 4]: h @ amat_a = Wh1 scores
    amat_b = np.einsum("hid,hd->ih", W, a2)  # [256, 4]
    wam = np.ascontiguousarray(
        np.concatenate([W_all, amat_a, amat_b], axis=1).astype(np.float32))
    ht = np.ascontiguousarray(h.T)
    wpt = np.ascontiguousarray(Wp.T)

    nc = _get_nc()
    in_maps = []
    for c in range(NCORES):
        qsl = slice(c * QN, (c + 1) * QN)
        in_maps.append({
            "ht": ht,
            "hqt": np.ascontiguousarray(ht[:, qsl]),
            "adjq": np.ascontiguousarray(adj[qsl, :].astype(np.int16)),
            "wam": wam,
            "wpt": wpt,
            "bp": bp,
        })

    res = run_bass_kernel_spmd(nc, in_maps, core_ids=list(range(NCORES)))
    global LAST_RESULTS
    LAST_RESULTS = res
    return np.concatenate([r["out"] for r in res.results], axis=0)

